# revision 3
# baseline (speedup 1.0000x reference)
"""Trainium2 Bass kernel for nn_KnowledgeBase (embedding_lookup).

Reference semantics (Q=3, R=128, V=64, EPS=0.4999999):
    saw      = (query - floor(query)) * R                      # [3]
    indices  = round(neighbor_map + saw)                       # [8,3]
    flat_idx = (indices.int32 @ [R^2, R, 1]) % R               # [8]
    values   = storage.reshape(R^3, V)[flat_idx]               # [8,64]
    weights  = sum(1 - |indices - saw|, axis=1)                # [8]
    out      = (values.T @ weights) / Q                        # [64]

Key static fact: flat_idx is taken mod R=128, and R^2 and R are multiples of
128, so flat_idx == indices[:, 2] mod 128 and only the first 128 rows of the
flattened table (storage[0, 0, :, :], 32KB of the 512MB input) are reachable
for ANY query.  The kernel therefore ships only that slice to the device.

Device kernel (per core, all 8 cores replicate the same tiny computation):
    aux DMA  : [8, 134] f32 = query (replicated), neighbor_map, iota 0..127
    table DMA: [128, 64] f32
    DVE chain computes saw / indices / weights / flat column index, builds a
    one-hot [8,128] via is_equal against the iota row, then two PE matmuls do
    gather + weighted reduction:  S[128,1] = onehot^T @ w ;  out = S^T @ T / 3.
"""

import os
import numpy as np

Q = 3
R = 128
V = 64
EPS = 0.4999999
NB = 8            # 2^Q neighbors
N_CORES = 8
AUX_COLS = 6 + R  # q(3) + nm(3) + iota(128)

_cache = {}


def _neighbor_map() -> np.ndarray:
    nm = [[]]
    for _ in range(Q):
        nm = [[EPS] + a for a in nm] + [[-EPS] + a for a in nm]
    return np.array(nm, dtype=np.float32)  # [8, 3]


def build_bass():
    """Build + compile the Bass module (cached)."""
    if "nc" in _cache:
        return _cache["nc"]

    import concourse.tile as tile
    from concourse import bacc, mybir

    f32 = mybir.dt.float32
    A = mybir.AluOpType

    nc = bacc.Bacc("TRN2", target_bir_lowering=False, debug=False)
    aux_d = nc.dram_tensor("aux", [NB, AUX_COLS], f32, kind="ExternalInput")
    tab_d = nc.dram_tensor("tab", [R, V], f32, kind="ExternalInput")
    out_d = nc.dram_tensor("out", [1, V], f32, kind="ExternalOutput")

    with tile.TileContext(nc) as tc:
        with (
            tc.tile_pool(name="sb", bufs=1) as pool,
            tc.tile_pool(name="ps", bufs=1, space="PSUM") as pp,
        ):
            aux = pool.tile([NB, AUX_COLS], f32)
            tab = pool.tile([R, V], f32)
            nc.sync.dma_start(aux[:], aux_d.ap())
            nc.sync.dma_start(tab[:], tab_d.ap())

            i32 = mybir.dt.int32
            q = aux[:, 0:Q]
            nm = aux[:, Q : 2 * Q]
            iota = aux[:, 2 * Q : 2 * Q + R]

            # floor(x) robust to ANY f32->i32 convert rounding mode (trunc,
            # RNE, away): r = cvt(x) lands on floor(x) or ceil(x); then
            # floor = r - (r > x).
            qi = pool.tile([NB, Q], i32)
            nc.vector.tensor_copy(qi[:], q)
            qf = pool.tile([NB, Q], f32)
            nc.vector.tensor_copy(qf[:], qi[:])
            gq = pool.tile([NB, Q], f32)
            nc.vector.tensor_tensor(gq[:], qf[:], q, A.is_gt)
            fl = pool.tile([NB, Q], f32)
            nc.vector.tensor_sub(fl[:], qf[:], gq[:])
            fr = pool.tile([NB, Q], f32)
            nc.vector.tensor_tensor(fr[:], q, fl[:], A.subtract)
            saw = pool.tile([NB, Q], f32)
            nc.vector.tensor_scalar_mul(saw[:], fr[:], 128.0)

            # indices = round(nm + saw) = floor(y + 0.5) here (verified against
            # jnp.round for the fixed test query; y + 0.5 > 0 always).
            y = pool.tile([NB, Q], f32)
            nc.vector.tensor_add(y[:], saw[:], nm)
            t = pool.tile([NB, Q], f32)
            nc.vector.tensor_scalar_add(t[:], y[:], 0.5)
            ti = pool.tile([NB, Q], i32)
            nc.vector.tensor_copy(ti[:], t[:])
            tf = pool.tile([NB, Q], f32)
            nc.vector.tensor_copy(tf[:], ti[:])
            gt = pool.tile([NB, Q], f32)
            nc.vector.tensor_tensor(gt[:], tf[:], t[:], A.is_gt)
            ind = pool.tile([NB, Q], f32)
            nc.vector.tensor_sub(ind[:], tf[:], gt[:])

            # weights = sum_j 1 - |ind - saw| : -|d| = min(d, -d), then fused
            # (add 1, accumulate) -> w [8,1]
            d = pool.tile([NB, Q], f32)
            nc.vector.tensor_sub(d[:], ind[:], saw[:])
            nd = pool.tile([NB, Q], f32)
            nc.vector.tensor_scalar_mul(nd[:], d[:], -1.0)
            a2 = pool.tile([NB, Q], f32)
            nc.vector.tensor_tensor(a2[:], d[:], nd[:], A.min)
            na = pool.tile([NB, Q], f32)
            w = pool.tile([NB, 1], f32)
            nc.vector.tensor_scalar(
                na[:], a2[:], 1.0, None, A.add, op1=A.add, accum_out=w[:]
            )

            # flat_idx mod 128 == ind[:,2] mod 128 (R^2, R are multiples of R;
            # exact in f32, everything < 2^24).  ind2 in [0,128] integral, so
            # mod 128 = ind2 - 128*(ind2 == 128).
            e = pool.tile([NB, 1], f32)
            nc.vector.tensor_scalar(
                e[:], ind[:, 2:3], 128.0, -128.0, A.is_equal, op1=A.mult
            )
            cf = pool.tile([NB, 1], f32)
            nc.vector.tensor_tensor(cf[:], ind[:, 2:3], e[:], A.add)

            # one-hot [8,128]: row i = (iota == flat_idx_i)
            onehot = pool.tile([NB, R], f32)
            nc.vector.tensor_scalar(onehot[:], iota, cf[:], None, A.is_equal)

            # S[p] = sum_i onehot[i,p] * w_i  (PE: K=8)
            ps1 = pp.tile([R, 1], f32)
            nc.tensor.matmul(ps1[:], onehot[:], w[:], start=True, stop=True)
            S = pool.tile([R, 1], f32)
            nc.vector.tensor_copy(S[:], ps1[:])

            # out[1,64] = S^T @ T  (PE: K=128), then / 3 (as *(1/3): DVE has
            # no divider; <=1ulp from reference's divide, well within tol)
            ps2 = pp.tile([1, V], f32)
            nc.tensor.matmul(ps2[:], S[:], tab[:], start=True, stop=True)
            outt = pool.tile([1, V], f32)
            nc.vector.tensor_scalar_mul(outt[:], ps2[:], 1.0 / float(Q))
            nc.sync.dma_start(out_d.ap(), outt[:])

    nc.compile()
    _cache["nc"] = nc
    return nc


def make_inputs(query: np.ndarray, storage: np.ndarray) -> dict:
    aux = np.zeros((NB, AUX_COLS), np.float32)
    aux[:, 0:Q] = np.asarray(query, np.float32)[None, :]
    aux[:, Q : 2 * Q] = _neighbor_map()
    aux[:, 2 * Q :] = np.arange(R, dtype=np.float32)[None, :]
    storage = np.asarray(storage, np.float32)
    tab = np.ascontiguousarray(storage.reshape(R**Q, V)[:R])
    return {"aux": aux, "tab": tab}


def run_on_hw(in_map: dict, trace: bool = False):
    from concourse.bass_utils import run_bass_kernel_spmd

    nc = build_bass()
    in_maps = [dict(in_map) for _ in range(N_CORES)]
    return run_bass_kernel_spmd(
        nc, in_maps, core_ids=list(range(N_CORES)), trace=trace
    )


def kernel(query: np.ndarray, storage: np.ndarray) -> np.ndarray:
    query = np.asarray(query, np.float32)
    storage = np.asarray(storage, np.float32)
    assert query.shape == (Q,), query.shape
    assert storage.shape == (R, R, R, V), storage.shape
    res = run_on_hw(make_inputs(query, storage), trace=False)
    return np.asarray(res.results[0]["out"], np.float32).reshape(V)
